# revision 5
# baseline (speedup 1.0000x reference)
"""MetaQuickSR Trainium2 kernel v2 (8-core SPMD, row-sharded).

Sharding: H=256 output-feature rows split 32/core (+4-row soft conv halo).
Each core: 4-layer CNN -> PE-transpose im2col cache -> Pos2Weight MLP ->
per-pixel locally-connected contraction on DVE (broadcast-mult + fold-tree)
-> PE-transposed, interleaved, contiguous writeback of its 64-row slab of
the (4,3,512,512) output.  No cross-core communication.
"""

import numpy as np
import ml_dtypes

import concourse.bass as bass
import concourse.mybir as mybir
from concourse.tile import TileContext
from concourse.bass_utils import run_bass_kernel_spmd
from concourse.masks import make_identity

BF16 = ml_dtypes.bfloat16

NCORES = 8
N, CI, Himg, Wimg, S = 4, 16, 256, 256, 2
ROWS = Himg // NCORES          # 32 output-feature rows per core
HALO = 4
NR = ROWS + 2 * HALO           # 40 buffered rows
WP = Wimg + 2                  # 258 zero-padded width
NPIX = ROWS * Wimg             # 8192 einsum pixels per core
NT = NPIX // 128               # 64 pixel tiles (t = 2*r0 + hf)
PCH = 8                        # 1024-pixel chunks per q plane
RGB_MEAN = (0.4488, 0.4371, 0.404)
RGB_RANGE = 255.0

_NC = None
_NC_KEY = None


def _legalize_waits(nc, lim=1):
    """This walrus build accepts only one sync-wait per instruction; move
    surplus waits onto same-engine NoOps inserted just before."""
    cnt = 0
    for f in nc.m.functions:
        for bb in f.blocks:
            new = []
            for inst in bb.instructions:
                si = inst.sync_info
                if si is not None and si.on_wait is not None \
                        and len(si.on_wait) > lim:
                    waits = list(si.on_wait)
                    excess, keep = waits[:-lim], waits[-lim:]
                    for w in excess:
                        cnt += 1
                        nop = mybir.InstNoOp(
                            name=f"I-lw{cnt}", opcode="NoOp",
                            engine=inst.engine, debug=inst.debug,
                            ins=[], outs=[],
                            sync_info=mybir.SyncInfo(on_wait=[w],
                                                     on_update=[]))
                        new.append(nop)
                        nc.inst_map[nop.name] = nop
                    inst.sync_info = mybir.SyncInfo(
                        on_wait=keep, on_update=list(si.on_update or []))
                new.append(inst)
            bb.instructions = new
    return cnt


def _build_program(use_b2=False, sim_safe=False):
    nc = bass.Bass(trn_type="TRN2")
    f32 = mybir.dt.float32
    bf = mybir.dt.bfloat16

    # packed constant inputs: [x | cw | w2p | cw0] bf16,
    # [w1 | cb | b1c | shift6 | b2p | ones] f32
    BFW = NR * WP + 4 * 9 * 16 + 16 + 256 + 144   # 11216
    FW = 4 + 2 + 6 + 432 + 128                    # 572
    f8 = mybir.dt.float8e4
    bfin = nc.dram_tensor("bfin", [128, BFW], bf, kind="ExternalInput")
    f32in = nc.dram_tensor("f32in", [128, FW], f32, kind="ExternalInput")
    f8in = nc.dram_tensor("f8in", [128, 2, 432], f8, kind="ExternalInput")
    post = nc.dram_tensor("post", [4, 3, NPIX], bf, kind="ExternalInput")
    outd = nc.dram_tensor("out", [4, 3, 2 * ROWS, 2 * Wimg], f32,
                          kind="ExternalOutput")
    # DRAM out viewed as [12 nc, 64 rows, 512 cols] (nc = 3n+c contiguous)
    outv = outd.rearrange("n c r w -> (n c) r w")

    mul, add = mybir.AluOpType.mult, mybir.AluOpType.add

    with TileContext(nc) as tc:
        with (
            tc.tile_pool(name="singles", bufs=1) as singles,
            tc.tile_pool(name="pos_p", bufs=6) as pos_p,
            tc.tile_pool(name="ht_p", bufs=10) as ht_p,
            tc.tile_pool(name="lws_p", bufs=24) as lws_p,
            tc.tile_pool(name="scr_p", bufs=1) as scr_p,
            tc.tile_pool(name="s2_p", bufs=1) as s2_p,
            tc.tile_pool(name="scr_p8", bufs=1) as scr_p8,
            tc.tile_pool(name="s2_p8", bufs=1) as s2_p8,
            tc.tile_pool(name="ow_p", bufs=2) as ow_p,
            tc.tile_pool(name="cps", bufs=1, space="PSUM") as cps,
            tc.tile_pool(name="tps", bufs=2, space="PSUM") as tps,
            tc.tile_pool(name="hps", bufs=1, space="PSUM") as hps,
            tc.tile_pool(name="lps", bufs=4, space="PSUM") as lps,
        ):
            # ---- resident inputs -------------------------------------
            bf_sb = singles.tile([128, BFW], bf)
            f32_sb = singles.tile([128, FW], f32)
            w2p8 = singles.tile([128, 2, 432], f8)
            fA = singles.tile([128, NR, WP], bf)
            fB = singles.tile([128, NR, WP], bf)
            ident = singles.tile([128, 128], bf)
            ident32 = singles.tile([128, 128], f32)
            # fT[hf][p=w-in-half, n, (r,kw), ci]: transposed l3 features.
            # einsum tile (r0,hf) reads the contiguous 144-elem window
            # starting at (3*r0)*16 of the flattened ((r,kw),ci) axis.
            fT = [singles.tile([128, 4, 102, 16], bf, name=f"fT{h}")
                  for h in range(2)]
            outq = [singles.tile([128, 12, NT], f32, name=f"outq{q}")
                    for q in range(4)]
            dummy = singles.tile([1, 16], bf)

            nc.scalar.dma_start(bf_sb[:, :], bfin[:, :])
            nc.scalar.dma_start(f32_sb[:, :], f32in[:, :])
            nc.scalar.dma_start(w2p8[:, :, :], f8in[:, :, :])
            for fz in (fA, fB):
                nc.gpsimd.memset(fz[:, 0:1, :], 0.0)
                nc.gpsimd.memset(fz[:, 39:40, :], 0.0)
                nc.gpsimd.memset(fz[:, 1:39, 0:1], 0.0)
                nc.gpsimd.memset(fz[:, 1:39, 257:258], 0.0)
            make_identity(nc, ident[:, :])
            make_identity(nc, ident32[:, :])

            # warm ACT's vector clock (1 wait per op) so conv relu-copies
            # only ever wait on PE.
            nc.scalar.copy(dummy[0:1, 0:1], bf_sb[0:1, 0:1])
            nc.scalar.copy(dummy[0:1, 1:2], f32_sb[0:1, 0:1])
            nc.scalar.copy(dummy[0:1, 2:3], fA[0:1, 0:1, 0:1])
            nc.scalar.copy(dummy[0:1, 3:4], fB[0:1, 0:1, 0:1])

            x_sb = bf_sb[:, 0:NR * WP].rearrange("p (r w) -> p r w", w=WP)
            cw_sb = bf_sb[:, NR * WP:NR * WP + 576].rearrange(
                "p (l t o) -> p l t o", t=9, o=16)
            cw0_sb = bf_sb[:, NR * WP + 576:NR * WP + 592]
            w1_sb = bf_sb[0:3, NR * WP + 592:NR * WP + 848]
            cwP_sb = bf_sb[:, NR * WP + 848:NR * WP + 992].rearrange(
                "p (l k o) -> p l k o", l=3, o=16)
            cb_sb = f32_sb[:, 0:4]
            b1_sb = f32_sb[:, 4:6]
            shift_sb = f32_sb[:, 6:12]
            b2p_sb = f32_sb[0:1, 12:444]
            ones_sb = f32_sb[0:1, 444:572]

            # ---- conv chain + interleaved im2col transposes ----------
            # l: 0:x->fA  1:fA->fB  2:fB->fA  3:fA->fB
            fins = [x_sb, fA, fB, fA]
            fouts = [fA, fB, fA, fB]
            tr_done = 0

            def emit_transposes(r_hi):
                """PE-transpose fB rows (+ACT gather-copy) for fT rows
                [tr_done, r_hi)."""
                nonlocal tr_done
                for r in range(tr_done, r_hi):
                    for hf in range(2):
                        for kw in range(3):
                            pst = tps.tile([128, 128], bf, tag="tps")
                            nc.tensor.transpose(
                                pst[:, :],
                                fB[:, r + 3, 128 * hf + kw:
                                   128 * hf + kw + 128],
                                ident[:, :])
                            pv = pst.rearrange("p (n c) -> p n c", c=32)
                            nc.scalar.activation(
                                fT[hf][:, :, 3 * r + kw, :],
                                pv[:, :, 0:16],
                                mybir.ActivationFunctionType.Copy)
                tr_done = r_hi

            def emit_conv_chunk(l, ch):
                fin, fout = fins[l], fouts[l]
                if True:
                    r0 = 1 + 2 * ch
                    ps = cps.tile([128, 2, 256], f32, tag="convps")
                    if l == 0:
                        # all 9 taps baked into the host-packed x copies:
                        # partitions 32n + 3*tap + ci, rows pre-shifted.
                        for n in range(4):
                            nc.tensor.matmul(
                                ps[32 * n:32 * n + 16, :, :],
                                cw0_sb[32 * n:32 * n + 27, :],
                                fin[32 * n:32 * n + 27,
                                    r0:r0 + 2, 1:257],
                                start=True, stop=True,
                                tile_position=(32 * n, 32 * n),
                                skip_group_check=True,
                            )
                    else:
                        # rows kh=0,1 paired via the +16-partition
                        # row-shifted feature copy (k=32); kh=2 single.
                        for kw in range(3):
                            for n in range(4):
                                nc.tensor.matmul(
                                    ps[32 * n:32 * n + 16, :, :],
                                    cwP_sb[32 * n:32 * n + 32, l - 1,
                                           kw, :],
                                    fin[32 * n:32 * n + 32,
                                        r0 - 1:r0 + 1, kw:kw + 256],
                                    start=(kw == 0), stop=False,
                                    tile_position=(32 * n, 32 * n),
                                    skip_group_check=True,
                                )
                        for kw in range(3):
                            for n in range(4):
                                nc.tensor.matmul(
                                    ps[32 * n:32 * n + 16, :, :],
                                    cw_sb[32 * n:32 * n + 16, l, 6 + kw, :],
                                    fin[32 * n:32 * n + 16,
                                        r0 + 1:r0 + 3, kw:kw + 256],
                                    start=False, stop=(kw == 2),
                                    tile_position=(32 * n, 32 * n),
                                    skip_group_check=True,
                                )
                    cscale = 1.0
                    if sim_safe:
                        # functional-sim only: avoid reading the psum
                        # partitions the quad matmuls never write
                        for n in range(4):
                            nc.scalar.activation(
                                fout[32 * n:32 * n + 16, r0:r0 + 2, 1:257],
                                ps[32 * n:32 * n + 16, :, :],
                                mybir.ActivationFunctionType.Relu,
                                bias=cb_sb[32 * n:32 * n + 16, l:l + 1],
                                scale=cscale)
                    else:
                        nc.scalar.activation(
                            fout[:, r0:r0 + 2, 1:257], ps[:, :, :],
                            mybir.ActivationFunctionType.Relu,
                            bias=cb_sb[:, l:l + 1], scale=cscale)
                    if l < 3:
                        # +16-partition row-shifted copy for the next
                        # layer's kh-pair matmuls
                        for n in range(4):
                            nc.gpsimd.dma_start(
                                out=fout[32 * n + 16:32 * n + 32,
                                         r0 - 1:r0 + 1, :],
                                in_=fout[32 * n:32 * n + 16,
                                         r0:r0 + 2, :])

            # ---- MLP + einsum helpers -------------------------------
            def emit_block_pair(qp, k):
                """Blocks (q=2qp, k) and (q=2qp+1, k): MLP + mults per
                4-tile block as before, but one shared fold chain over
                all 8 tiles to halve DVE op overhead."""
                scr = scr_p.tile([128, 8, 12, 144], bf, tag="scr")
                for b2 in range(2):
                    q = 2 * qp + b2
                    pos_t = pos_p.tile([3, 512], bf, tag="pos")
                    nc.sync.dma_start(
                        pos_t[:, :], post[q, :, 512 * k:512 * (k + 1)])
                    hT = ht_p.tile([128, 2, 512], f8, tag="ht")
                    for jh in range(2):
                        hp = hps.tile([128, 512], f32, tag="hps")
                        nc.tensor.matmul(
                            hp[:, :],
                            w1_sb[:, jh * 128:(jh + 1) * 128],
                            pos_t[:, :],
                            start=True, stop=True)
                        nc.scalar.activation(
                            hT[:, jh, :], hp[:, :],
                            mybir.ActivationFunctionType.Relu,
                            bias=b1_sb[:, jh:jh + 1], scale=1.0)
                    for tb in range(4):
                        t = 4 * k + tb
                        r0, hf = t // 2, t % 2
                        lwp = lps.tile([128, 3, 144], f32, tag="lwp")
                        nc.tensor.matmul(
                            lwp[:, :, :],
                            hT[:, :, tb * 128:(tb + 1) * 128],
                            w2p8[:, :, :],
                            start=True, stop=(not use_b2),
                            perf_mode=mybir.MatmulPerfMode.DoubleRow)
                        if use_b2:
                            nc.tensor.matmul(
                                lwp[:, :, :], ones_sb[:, :],
                                b2p_sb[:, :].rearrange(
                                    "p (c s) -> p c s", s=144),
                                start=False, stop=True)
                        lws = lws_p.tile([128, 3, 144], bf, tag="lws")
                        nc.scalar.activation(
                            lws[:, :, :], lwp[:, :, :],
                            mybir.ActivationFunctionType.Copy,
                            scale=1.0 / 16.0)
                        # einsum: scr[p,n,c,s] = fT[p,n,s] * lws[p,c,s]
                        ftw = fT[hf].rearrange("p n r c -> p n (r c)")[
                            :, :, 48 * r0:48 * r0 + 144]
                        nc.vector.tensor_tensor(
                            out=scr[:, 4 * b2 + tb].rearrange(
                                "p (n c) s -> p n c s", c=3),
                            in0=ftw[:, :, None, :].broadcast_to(
                                (128, 4, 3, 144)),
                            in1=lws[:, None, :, :].broadcast_to(
                                (128, 4, 3, 144)),
                            op=mul)
                t = 4 * k + 3
                s2 = s2_p.tile([128, 8, 12, 72], bf, tag="s2")
                nc.vector.tensor_tensor(
                    out=s2[:, :, :, :],
                    in0=scr[:, :, :, 0:72],
                    in1=scr[:, :, :, 72:144], op=add)
                nc.vector.tensor_tensor(
                    out=s2[:, :, :, 0:36],
                    in0=s2[:, :, :, 0:36],
                    in1=s2[:, :, :, 36:72], op=add)
                nc.vector.tensor_tensor(
                    out=s2[:, :, :, 0:18],
                    in0=s2[:, :, :, 0:18],
                    in1=s2[:, :, :, 18:36], op=add)
                nc.vector.tensor_tensor(
                    out=s2[:, :, :, 0:9],
                    in0=s2[:, :, :, 0:9],
                    in1=s2[:, :, :, 9:18], op=add)
                for b2 in range(2):
                    q = 2 * qp + b2
                    nc.vector.tensor_reduce(
                        out=outq[q][:, :, t - 3:t + 1].rearrange(
                            "p c t -> p t c"),
                        in_=s2[:, 4 * b2:4 * b2 + 4, :, 0:9],
                        axis=mybir.AxisListType.X, op=add)

            def emit_writeback(si):
                # outq[q] [128=w, 12*64 cols] -> 6 col-blocks of 128;
                # psumT partitions = (ncl, r0, hf), free = w.
                for b in range(6):
                    psts = []
                    for sj in range(2):
                        q = 2 * si + sj
                        pst = tps.tile([128, 128], f32, tag="tps")
                        oqf = outq[q].rearrange("p a b -> p (a b)")
                        nc.tensor.transpose(
                            pst[:, :], oqf[:, 128 * b:128 * b + 128],
                            ident32[:, :])
                        psts.append(pst)
                    outw = ow_p.tile([128, 256], f32, tag="ow")
                    owv = outw.rearrange("p (w s) -> p w s", s=2)
                    for sj in range(2):
                        nc.scalar.activation(
                            owv[:, :, sj], psts[sj][:, :],
                            mybir.ActivationFunctionType.Identity,
                            bias=shift_sb[:, b:b + 1], scale=1.0)
                    dst = outv[2 * b:2 * b + 2].rearrange(
                        "e (r s1) (hf wsj) -> e r hf s1 wsj",
                        s1=2, hf=2)[:, :, :, si, :]
                    nc.sync.dma_start(out=dst, in_=outw[:, :])

            blocks_done = 0
            for w in range(26):
                for l in range(4):
                    ch = w - l
                    if 0 <= ch < 19:
                        emit_conv_chunk(l, ch)
                ch3 = w - 3
                if ch3 >= 2:
                    emit_transposes(min(2 * ch3, 34))
                kmax = min(16, max(0, ch3 - 1))
                while blocks_done < kmax:
                    for qp in range(2):
                        emit_block_pair(qp, blocks_done)
                    blocks_done += 1
            emit_transposes(34)
            while blocks_done < 16:
                for qp in range(2):
                    emit_block_pair(qp, blocks_done)
                blocks_done += 1
            emit_writeback(0)
            emit_writeback(1)

            # (emit_writeback defined above, invoked from the driver)
    _legalize_waits(nc)
    return nc


def _get_nc(use_b2):
    global _NC, _NC_KEY
    if _NC is None or _NC_KEY != use_b2:
        _NC = _build_program(use_b2=use_b2)
        _NC_KEY = use_b2
    return _NC


def _prep_inputs(x, pos_mat, c0w, c0b, c1w, c1b, c2w, c2b, c3w, c3b,
                 w1, b1, w2, b2):
    """Host-side packing of per-core input dicts."""
    x = np.asarray(x, np.float32)
    pos = np.asarray(pos_mat, np.float32).reshape(-1, 3)

    # conv weights: cw[32n+ci, l, kh*3+kw, co]
    cwp = np.zeros((128, 4, 9, 16), np.float32)
    cbp = np.zeros((128, 4), np.float32)
    for l, (wl, bl) in enumerate(((c0w, c0b), (c1w, c1b),
                                  (c2w, c2b), (c3w, c3b))):
        wl = np.asarray(wl, np.float32)          # (co, ci, 3, 3)
        K = wl.shape[1]
        t = wl.transpose(1, 2, 3, 0).reshape(K, 9, 16)   # (ci, tap, co)
        for n in range(4):
            cwp[32 * n:32 * n + K, l] = t
            cbp[32 * n:32 * n + 16, l] = np.asarray(bl, np.float32)

    F8 = ml_dtypes.float8_e4m3
    w1 = np.asarray(w1, np.float32)              # (3, 256)
    b1p = np.asarray(b1, np.float32).reshape(2, 128).T.copy()  # [j, jh]

    # w2 columns: orig (s=ci*9+tap, c) -> permuted (c, tap, ci); x16 for fp8
    w2 = np.asarray(w2, np.float32).reshape(256, 16, 9, 3)     # j, ci, tap, c
    w2pm = w2.transpose(0, 3, 2, 1).reshape(256, 432)          # j,(c,tap,ci)
    w2p8 = (w2pm.reshape(2, 128, 432) * 16.0).astype(F8)       # [jh, j, 432]
    w2p8 = np.ascontiguousarray(w2p8.transpose(1, 0, 2))       # [j, jh, 432]
    b2 = np.asarray(b2, np.float32).reshape(16, 9, 3)
    b2pk = b2.transpose(2, 1, 0).reshape(1, 432) * 16.0        # (c, tap, ci)

    # pos rows ordered (h, si, w, sj); per-core chunk -> (q, 3, NPIX)
    posr = pos.reshape(Himg, 2, Wimg, 2, 3)

    # per-(partition, block) mean shift: nc = 2*b + (p >= 64), c = nc % 3
    shift6 = np.zeros((128, 6), np.float32)
    for b in range(6):
        for p in range(128):
            ncidx = 2 * b + (1 if p >= 64 else 0)
            shift6[p, b] = RGB_RANGE * RGB_MEAN[ncidx % 3]

    # f32 pack: [cb | b1c | shift6 | b2p | ones]
    FW = 4 + 2 + 6 + 432 + 128
    f32pk = np.zeros((128, FW), np.float32)
    f32pk[:, 0:4] = cbp
    f32pk[:, 4:6] = b1p
    f32pk[:, 6:12] = shift6
    f32pk[0, 12:444] = b2pk[0]
    f32pk[0, 444:572] = 1.0
    w1pk = np.zeros((128, 256), np.float32)
    w1pk[0:3] = w1

    # row-pair weights for layers 1-3: cwP[32n + 16*g + ci, l-1, kw, co]
    # = c{l}w[co, ci, kh=g, kw]
    cwP = np.zeros((128, 3, 3, 16), np.float32)
    for li, wl in enumerate((c1w, c2w, c3w)):
        wl = np.asarray(wl, np.float32)          # (co, ci, 3, 3)
        for g in range(2):
            t = wl[:, :, g, :].transpose(1, 2, 0)    # (ci, kw, co)
            for n in range(4):
                cwP[32 * n + 16 * g:32 * n + 16 * g + 16, li] = t

    # layer-0 weights with all 9 taps on the contraction axis:
    # cw0[32n + 3*tap + ci, co] = c0w[co, ci, kh, kw]
    c0wf = np.asarray(c0w, np.float32)            # (16, 3, 3, 3)
    cw0 = np.zeros((128, 16), np.float32)
    t0 = c0wf.transpose(2, 3, 1, 0).reshape(27, 16)   # (kh,kw,ci),co
    for n in range(4):
        cw0[32 * n:32 * n + 27] = t0

    # x pre-shifted for the 9 taps: xpad gives zero-padding on every side
    OFF, COFF = 6, 2
    xpad = np.zeros((4, 3, Himg + 2 * OFF, Wimg + 2 * COFF), np.float32)
    xpad[:, :, OFF:OFF + Himg, COFF:COFF + Wimg] = x

    in_maps = []
    for core in range(NCORES):
        h0 = core * ROWS
        xh = np.zeros((128, NR, WP), np.float32)
        for n in range(4):
            for tap in range(9):
                kh, kw = tap // 3, tap % 3
                # xh[32n+3t+ci, r, w] = x[n, ci, h0-5+r+kh, w-2+kw]
                xh[32 * n + 3 * tap:32 * n + 3 * tap + 3] = \
                    xpad[n, :, OFF + h0 - 5 + kh:
                         OFF + h0 - 5 + kh + NR,
                         kw:kw + WP]
        bfpk = np.concatenate(
            [xh.reshape(128, -1), cwp.reshape(128, -1), cw0, w1pk,
             cwP.reshape(128, -1)], axis=1)
        pc = posr[h0:h0 + ROWS].transpose(1, 3, 4, 0, 2)  # si,sj,3,h,w
        pc = pc.reshape(2, 2, 3, NPIX).reshape(4, 3, NPIX)
        in_maps.append({
            "bfin": bfpk.astype(BF16),
            "f32in": f32pk,
            "f8in": w2p8,
            "post": np.ascontiguousarray(pc.astype(BF16)),
        })
    return in_maps


LAST_RESULTS = None
TRACE = False


def kernel(**inputs):
    global LAST_RESULTS
    use_b2 = bool(np.any(np.asarray(inputs["b2"], np.float32)))
    nc = _get_nc(use_b2)
    in_maps = _prep_inputs(**inputs)
    res = run_bass_kernel_spmd(nc, in_maps, core_ids=list(range(NCORES)),
                               trace=TRACE)
    LAST_RESULTS = res
    out = np.concatenate([res.results[i]["out"] for i in range(NCORES)],
                         axis=2)
    return out.astype(np.float32)
